# revision 30
# baseline (speedup 1.0000x reference)
import numpy as np

import concourse.bass as bass
import concourse.bacc as bacc
import concourse.mybir as mybir
import concourse.tile as tile
from concourse.bass_utils import run_bass_kernel_spmd

F16 = np.float16
F32 = mybir.dt.float32
BF = mybir.dt.float16

B = 8
T = 1024
E = 768
H = 12
DH = 64
HD1 = DH + 1  # head dim + ones column for softmax denominator
NE = E // 128  # 6 partition tiles along embed dim
NT = T // 128  # 8 partition tiles along seq dim
NP = H // 2  # 6 head pairs (pair p = heads 2p, 2p+1 living in qhT/khT[p])


def _ldw_sig(inst):
    return (
        str(inst.ins[0]),
        str(inst.tile_position),
        str(inst.tile_size),
        str(inst.perf_mode),
        str(inst.is_transpose),
    )


def _row_range(inst):
    tp = inst.tile_position
    ts = inst.tile_size
    r0 = tp[0] if tp else 0
    rs = ts[0] if ts else 128
    return (r0, r0 + rs)


def _elide_redundant_ldweights(nc):
    """Drop Ldweights whose weights AP matches the last load into the same PE
    row range, with no overlapping load in between (matmults carry
    ldweights=False post-legalize, so walrus reuses the PE array contents).
    Tracked per row-group so row-tiled matmul pairs can ping-pong without
    reloading. Waits/deps of dropped loads move to the next PE instruction."""
    removed = 0
    for b in nc.main_func.blocks:
        insts = list(b.instructions)
        keep = []
        last = {}  # (row0, row1) -> sig
        pending = None
        for inst in insts:
            if isinstance(inst, mybir.InstLdweights):
                rr = _row_range(inst)
                s = _ldw_sig(inst)
                if last.get(rr) == s:
                    pending = inst
                    removed += 1
                    continue
                # invalidate overlapping row ranges
                for k in [k for k in last if not (k[1] <= rr[0] or k[0] >= rr[1])]:
                    del last[k]
                last[rr] = s
            elif isinstance(inst, mybir.InstMatmult):
                if pending is not None:
                    si = pending.sync_info
                    if si is not None and (len(si.on_wait) or len(si.on_update)):
                        mi = inst.sync_info
                        ow = list(si.on_wait)
                        ou = list(si.on_update)
                        if mi is not None:
                            ow = list(mi.on_wait) + ow
                            ou = list(mi.on_update) + ou
                        inst.sync_info = mybir.SyncInfo(on_wait=ow, on_update=ou)
                    inst.merge_dependencies_from(pending)
                    pending = None
            elif getattr(inst, "engine", None) == mybir.EngineType.PE:
                last.clear()
                if pending is not None:
                    inst.merge_dependencies_from(pending)
                    pending = None
            keep.append(inst)
        if len(keep) != len(insts):
            del b.instructions[:]
            b.instructions.extend(keep)
    return removed


def _build():
    nc = bacc.Bacc("TRN2", target_bir_lowering=False, debug=False)

    qT = nc.declare_dram_parameter("qT", [E, T], BF, isOutput=False)
    kT = nc.declare_dram_parameter("kT", [E, T], BF, isOutput=False)
    vT = nc.declare_dram_parameter("vT", [E, T], BF, isOutput=False)
    WqT = nc.declare_dram_parameter("WqT", [E, E], BF, isOutput=False)
    WkT = nc.declare_dram_parameter("WkT", [E, E], BF, isOutput=False)
    WvT = nc.declare_dram_parameter("WvT", [E, E], BF, isOutput=False)
    WoT = nc.declare_dram_parameter("WoT", [E, E], BF, isOutput=False)
    selD = nc.declare_dram_parameter("selD", [97, 384], BF, isOutput=False)
    out = nc.declare_dram_parameter("out", [T, E], F32, isOutput=True)

    EXP = mybir.ActivationFunctionType.Exp

    with tile.TileContext(nc) as tc:
        with (
            tc.tile_pool(name="persist", bufs=1) as pp,
            tc.tile_pool(name="xin", bufs=2) as xp,
            tc.tile_pool(name="w", bufs=2) as wp,
            tc.tile_pool(name="exps", bufs=2) as ep,
            tc.tile_pool(name="dn", bufs=1) as dn,
            tc.tile_pool(name="ob", bufs=2) as op,
            tc.tile_pool(name="pmm", bufs=1, space="PSUM") as pmm,
            tc.tile_pool(name="pscore", bufs=1, space="PSUM") as psc,
            tc.tile_pool(name="pctx", bufs=2, space="PSUM") as pcx,
        ):
            # ---- persistent sbuf tensors ----
            qhT = [pp.tile([128, T], BF, name=f"qhT{i}") for i in range(NE)]
            khT = [pp.tile([128, T], BF, name=f"khT{i}") for i in range(NE)]
            vh1 = [pp.tile([128, H * HD1], BF, name=f"vh1_{i}") for i in range(NT)]
            # mgP[p]: unnormalized ctx (heads 2p rows 0-63 / 2p+1 rows 64-127),
            # normalized IN PLACE before the output projection.
            mgP = [pp.tile([128, T], BF, name=f"mgP{p}") for p in range(NE)]
            sel = pp.tile([97, 384], BF, name="sel")
            scrA = pp.tile([97, 512], F32, name="scrA")
            scrB = pp.tile([33, 512], F32, name="scrB")
            # keep-warm: dummy matmuls fill DMA-wait gaps in the first ~35us
            # so the HAM clock gate never re-throttles the PE (idle >3.4us
            # drops it to half clock; early matmuls measured 375-584ns vs
            # 213ns warm). Results land in a junk psum tile nobody reads.
            dumW = pp.tile([128, 64], BF, name="dumW")
            dumR = pp.tile([128, 512], BF, name="dumR")
            nc.vector.memset(dumW[:], 0.0)
            nc.vector.memset(dumR[:], 0.0)
            dumP = pcx.tile([HD1, 512], F32, tag="ctx", name="dumP")

            def warm(n):
                for _ in range(n):
                    nc.tensor.matmul(
                        dumP[0:64, :], dumW[:], dumR[:],
                        start=True, stop=True, skip_group_check=True,
                    )
            # den/rcp tiles are shared between head groups g=0/1 via a bufs=1
            # pool: group 1's memset WAR-waits on group 0's last reader.
            _den_cache = {}

            def get_den(g):
                if g not in _den_cache:
                    dA = [
                        dn.tile([97, 512], F32, tag=f"dA{qb}", name=f"dA{g}_{qb}")
                        for qb in range(2)
                    ]
                    dB = [
                        dn.tile([33, 512], F32, tag=f"dB{qb}", name=f"dB{g}_{qb}")
                        for qb in range(2)
                    ]
                    rA = [
                        dn.tile([97, 512], BF, tag=f"rA{qb}", name=f"rA{g}_{qb}")
                        for qb in range(2)
                    ]
                    rB = [
                        dn.tile([33, 512], BF, tag=f"rB{qb}", name=f"rB{g}_{qb}")
                        for qb in range(2)
                    ]
                    for qb in range(2):
                        nc.vector.memset(dA[qb][:], 1.0)
                        nc.vector.memset(dB[qb][:], 1.0)
                    _den_cache[g] = (dA, dB, rA, rB)
                return _den_cache[g]

            def dma_in_chunks(dst, src, nch=2, eng=None):
                # each dma_start costs ~0.5us on its issuing sequencer, so
                # chunk sparingly and spread issues across SP and Activation
                eng = eng or nc.sync
                p = dst.shape[0]
                step = p // nch
                for c in range(nch):
                    eng.dma_start(
                        dst[c * step : (c + 1) * step, :],
                        src[c * step : (c + 1) * step, :],
                    )

            def load6(dram, pool, tag_prefix, cols, nch=1, eng=None):
                ts = []
                for i in range(NE):
                    t_ = pool.tile(
                        [128, cols], BF, tag=f"{tag_prefix}{i}", name=f"{tag_prefix}{i}"
                    )
                    dma_in_chunks(t_, dram[i * 128 : (i + 1) * 128, :], nch, eng)
                    ts.append(t_)
                return ts

            # ---- upfront DMA issue (interleaved so first-needed lands
            # first; weights on the sync sequencer, activations on scalar)
            xtq, wtq, xtk, wtk = [], [], [], []
            for i in range(NE):
                nch = 2
                w_ = wp.tile([128, E], BF, tag=f"w{i}", name=f"wq{i}")
                dma_in_chunks(w_, WqT[i * 128 : (i + 1) * 128, :], nch, nc.sync)
                wtq.append(w_)
                x_ = xp.tile([128, T], BF, tag=f"x{i}", name=f"xq{i}")
                dma_in_chunks(x_, qT[i * 128 : (i + 1) * 128, :], nch, nc.scalar)
                xtq.append(x_)
                w2 = wp.tile([128, E], BF, tag=f"w{i}", name=f"wk{i}")
                dma_in_chunks(w2, WkT[i * 128 : (i + 1) * 128, :], nch, nc.sync)
                wtk.append(w2)
                x2 = xp.tile([128, T], BF, tag=f"x{i}", name=f"xk{i}")
                dma_in_chunks(x2, kT[i * 128 : (i + 1) * 128, :], nch, nc.scalar)
                xtk.append(x2)
            # V inputs/weights get their own tags: they are consumed (during
            # head pairs 0-1) long before the last Q/K-proj matmuls that a
            # shared-tag WAR dependency would wait on.
            xv = load6(vT, pp, "xv", T)
            wv = load6(WvT, pp, "wv", E)
            nc.sync.dma_start(sel[:], selD[:, :])
            for tt in range(NT):
                v_ = vh1[tt][:].rearrange("p (h d) -> p h d", d=HD1)
                nc.vector.memset(v_[:, :, DH:HD1], 1.0)

            # ---- emission helpers (generators yield (ns_estimate) per chunk)
            def proj_qk(xt, wt, dst, oc, nwarm=0):
                # dst[oc][o, t] = sum_i W[i, o]^T x[i, t]; i-outer so each
                # weight block is loaded once and serves both 512-col halves
                ps = pmm.tile([128, T], F32, tag="mm", name=f"pj{oc}")
                for i in range(NE):
                    warm(nwarm)
                    for half in range(2):
                        c0 = half * 512
                        nc.tensor.matmul(
                            ps[:, c0 : c0 + 512],
                            wt[i][:, oc * 128 : (oc + 1) * 128],
                            xt[i][:, c0 : c0 + 512],
                            start=(i == 0),
                            stop=(i == NE - 1),
                            skip_group_check=True,
                        )
                    if i == 2:
                        yield 1600
                # split the drain copy across DVE and the idle GpSimd so the
                # single-buffered psum frees fast
                nc.vector.tensor_copy(dst[oc][:, 0:512], ps[:, 0:512])
                nc.vector.tensor_copy(dst[oc][:, 512:1024], ps[:, 512:1024])
                yield 1600

            def proj_v(tt, nwarm=0):
                # vh[t, (h d)] = sum_i vT[i, t]^T WvT[i, (h d)]
                v_ = vh1[tt][:].rearrange("p (h d) -> p h d", d=HD1)
                ps = pmm.tile([128, E], F32, tag="mm", name=f"pv{tt}")
                for i in range(NE):
                    warm(nwarm)
                    for half, cw in ((0, 512), (1, 256)):
                        c0 = half * 512
                        nc.tensor.matmul(
                            ps[:, c0 : c0 + cw],
                            xv[i][:, tt * 128 : (tt + 1) * 128],
                            wv[i][:, c0 : c0 + cw],
                            start=(i == 0),
                            stop=(i == NE - 1),
                            skip_group_check=True,
                        )
                    if i == 2:
                        yield 1300
                nc.vector.tensor_copy(
                    v_[:, 0:6, 0:DH],
                    ps[:, 0:384].rearrange("p (h d) -> p h d", d=DH),
                )
                nc.vector.tensor_copy(
                    v_[:, 6:H, 0:DH],
                    ps[:, 384:E].rearrange("p (h d) -> p h d", d=DH),
                )
                yield 1300

            def scores_round(p, kt, esA, esB):
                # paired row-tiled scores: head 2p in PE rows 0-63,
                # head 2p+1 in rows 64-127, concurrent per qb; one exp
                # instruction per head over the full 1024 queries.
                pa = psc.tile([128, T], F32, tag="sA", name=f"sA{p}_{kt}")
                pb = psc.tile([128, T], F32, tag="sB", name=f"sB{p}_{kt}")
                for qb in range(2):
                    c0 = qb * 512
                    nc.tensor.matmul(
                        pa[:, c0 : c0 + 512],
                        khT[p][0:DH, kt * 128 : (kt + 1) * 128],
                        qhT[p][0:DH, c0 : c0 + 512],
                        start=True,
                        stop=True,
                        skip_group_check=True,
                    )
                    nc.tensor.matmul(
                        pb[:, c0 : c0 + 512],
                        khT[p][DH:128, kt * 128 : (kt + 1) * 128],
                        qhT[p][DH:128, c0 : c0 + 512],
                        start=True,
                        stop=True,
                        skip_group_check=True,
                    )
                nc.scalar.activation(esA[kt][:], pa[:], EXP, scale=0.125)
                nc.scalar.activation(esB[kt][:], pb[:], EXP, scale=0.125)

            def ctx_head(h, es, alt_psum=False):
                # unnormalized ctx + denominator via the ones column, yields per
                # kt. alt_psum borrows the (then-idle) score psum banks so two
                # epilogue heads can interleave without WAR head-of-line blocks.
                if alt_psum:
                    pcs = [
                        psc.tile([HD1, 512], F32, tag=("sA" if qb == 0 else "sB"),
                                 name=f"pc{h}_{qb}")
                        for qb in range(2)
                    ]
                else:
                    pcs = [
                        pcx.tile([HD1, 512], F32, tag="ctx", name=f"pc{h}_{qb}")
                        for qb in range(2)
                    ]
                for kt in range(NT):
                    for qb in range(2):
                        nc.tensor.matmul(
                            pcs[qb][:],
                            vh1[kt][:, h * HD1 : (h + 1) * HD1],
                            es[kt][:, qb * 512 : (qb + 1) * 512],
                            start=(kt == 0),
                            stop=(kt == NT - 1),
                            skip_group_check=True,
                        )
                    yield 550
                g, r = h // 6, h % 6
                p2, half = h // 2, h % 2
                dA, dB, rA, rB = get_den(g)
                dent = dA if r < 4 else dB
                drow = 32 * r if r < 4 else 32 * (r - 4)
                for qb in range(2):
                    nc.vector.tensor_copy(
                        mgP[p2][half * DH : (half + 1) * DH, qb * 512 : (qb + 1) * 512],
                        pcs[qb][0:DH, :],
                    )
                    nc.vector.tensor_copy(
                        dent[qb][drow : drow + 1, :], pcs[qb][DH:HD1, :]
                    )
                if r == 3:
                    for qb in range(2):
                        recip(rA[qb], dA[qb], scrA)
                if r == 4:
                    norm_pairs(g, (0, 1))
                if r == 5:
                    for qb in range(2):
                        recip(rB[qb], dB[qb], scrB)
                    norm_pairs(g, (2,))
                yield 800

            def recip(dst, den_t, scr):
                nc.vector.reciprocal_approx_fast(scr[:], den_t[:])
                nc.vector.tensor_copy(dst[:], scr[:])

            def norm_pairs(g, js):
                # broadcast 1/den to 64 rows/head via PE, normalize mgP in place
                _, _, rA, rB = get_den(g)
                for j in js:
                    p = g * 3 + j
                    for qb in range(2):
                        bcps = pmm.tile([128, 512], F32, tag="mm", name=f"bc{p}_{qb}")
                        if j < 2:
                            lhsT = sel[0:97, j * 128 : (j + 1) * 128]
                            rhs = rA[qb][:]
                        else:
                            lhsT = sel[0:33, 256:384]
                            rhs = rB[qb][:]
                        nc.tensor.matmul(bcps[:], lhsT, rhs, start=True, stop=True)
                        nc.vector.tensor_mul(
                            mgP[p][:, qb * 512 : (qb + 1) * 512],
                            mgP[p][:, qb * 512 : (qb + 1) * 512],
                            bcps[:],
                        )

            # ---- the interleaved schedule ----
            # filler generators consumed a few PE-chunks per score round
            def chain(gens):
                for gg in gens:
                    yield from gg

            esd = {}

            def es_tiles(p):
                # bufs=2 rotation: pair p and p-2 share a buffer. ctx(p-2) is
                # fully emitted during pair p-1 (lag-1 staggering), so the WAR
                # dep of exp(p) on ctx(p-2) points backward in program order.
                A = [
                    ep.tile([128, T], BF, tag=f"eA{kt}", name=f"eA{p}_{kt}")
                    for kt in range(NT)
                ]
                Bt = [
                    ep.tile([128, T], BF, tag=f"eB{kt}", name=f"eB{p}_{kt}")
                    for kt in range(NT)
                ]
                return A, Bt

            # prologue: only pair 0's projections; everything else is filler
            warm(8)
            for gen in (
                proj_qk(xtq, wtq, qhT, 0, nwarm=2),
                proj_qk(xtk, wtk, khT, 0, nwarm=2),
            ):
                for _ in gen:
                    pass

            for p in range(NP):
                esA, esB = es_tiles(p)
                esd[2 * p] = esA
                esd[2 * p + 1] = esB
                # DMA-independent work (projections of already-loaded Q/K)
                # leads each chain; V waits for its own late-arriving DMAs.
                gens = []
                nw = 2 if p == 0 else 0
                if p + 1 < NP:
                    gens.append(proj_qk(xtq, wtq, qhT, p + 1, nwarm=nw))
                    gens.append(proj_qk(xtk, wtk, khT, p + 1, nwarm=nw))
                if p == 0:
                    gens.extend(proj_v(tt, nwarm=1) for tt in range(5))
                elif p == 1:
                    gens.extend(proj_v(tt) for tt in range(5, NT))
                if p > 0:
                    hA, hB = 2 * (p - 1), 2 * (p - 1) + 1
                    gens.append(ctx_head(hA, esd[hA]))
                    gens.append(ctx_head(hB, esd[hB]))
                fil = chain(gens)
                for kt in range(NT):
                    scores_round(p, kt, esA, esB)
                    # drain ~one ACT-round worth of filler to keep the PE
                    # queue deep while the exps grind
                    budget = 2100
                    while fil is not None and budget > 0:
                        try:
                            budget -= next(fil)
                        except StopIteration:
                            fil = None
                if fil is not None:
                    for _ in fil:
                        pass

            # epilogue: pair 5's two ctx heads interleave, accumulating in
            # different psum pools so the PE can stream both concurrently
            g10 = ctx_head(2 * (NP - 1), esd[2 * (NP - 1)])
            g11 = ctx_head(2 * (NP - 1) + 1, esd[2 * (NP - 1) + 1], alt_psum=True)
            alive = [g10, g11]
            while alive:
                for gg in list(alive):
                    try:
                        next(gg)
                    except StopIteration:
                        alive.remove(gg)

            # Wo load late: reuses w{i} tags once K-projections are done
            wo = load6(WoT, wp, "w", E)

            # ---- output projection (double-buffered via the idle score
            # psum banks; i-outer so each mgP block loads once) ----
            for tt in range(NT):
                ob = op.tile([128, E], F32, tag="ob", name=f"ob{tt}")
                po = psc.tile(
                    [128, E], F32, tag=("sA" if tt % 2 == 0 else "sB"), name=f"po{tt}"
                )
                for p in range(NE):
                    for half, cw in ((0, 512), (1, 256)):
                        c0 = half * 512
                        nc.tensor.matmul(
                            po[:, c0 : c0 + cw],
                            mgP[p][:, tt * 128 : (tt + 1) * 128],
                            wo[p][:, c0 : c0 + cw],
                            start=(p == 0),
                            stop=(p == NE - 1),
                            skip_group_check=True,
                        )
                nc.vector.tensor_copy(ob[:], po[:])
                for c in range(2):
                    nc.sync.dma_start(
                        out[tt * 128 + c * 64 : tt * 128 + (c + 1) * 64, :],
                        ob[c * 64 : (c + 1) * 64, :],
                    )

    _elide_redundant_ldweights(nc)
    nc.finalize()
    return nc


_NC = None
TRACE = False
LAST_RESULT = None


def _get_nc():
    global _NC
    if _NC is None:
        _NC = _build()
    return _NC


def kernel(**inputs):
    q = np.asarray(inputs["q"], dtype=np.float32)
    k = np.asarray(inputs["k"], dtype=np.float32)
    v = np.asarray(inputs["v"], dtype=np.float32)
    w = {
        n: np.ascontiguousarray(np.asarray(inputs[n], dtype=np.float32).T).astype(F16)
        for n in ("Wq", "Wk", "Wv", "Wo")
    }
    sel = np.zeros((97, 384), dtype=F16)
    for j in range(3):
        sel[(32 * 2 * j) % 128, j * 128 : j * 128 + 64] = 1.0
        sel[(32 * (2 * j + 1)) % 128, j * 128 + 64 : (j + 1) * 128] = 1.0

    nc = _get_nc()
    in_maps = []
    for b in range(B):
        in_maps.append({
            "qT": np.ascontiguousarray(q[b].T).astype(F16),
            "kT": np.ascontiguousarray(k[b].T).astype(F16),
            "vT": np.ascontiguousarray(v[b].T).astype(F16),
            "WqT": w["Wq"],
            "WkT": w["Wk"],
            "WvT": w["Wv"],
            "WoT": w["Wo"],
            "selD": sel,
        })
    res = run_bass_kernel_spmd(nc, in_maps, list(range(B)), trace=TRACE)
    global LAST_RESULT
    LAST_RESULT = res
    return np.stack(
        [np.asarray(res.results[b]["out"], dtype=np.float32) for b in range(B)], axis=0
    )


# revision 33
# speedup vs baseline: 1.0116x; 1.0116x over previous
import numpy as np

import concourse.bass as bass
import concourse.bacc as bacc
import concourse.mybir as mybir
import concourse.tile as tile
from concourse.bass_utils import run_bass_kernel_spmd

F16 = np.float16
F32 = mybir.dt.float32
BF = mybir.dt.float16

B = 8
T = 1024
E = 768
H = 12
DH = 64
HD1 = DH + 1  # head dim + ones column for softmax denominator
NE = E // 128  # 6 partition tiles along embed dim
NT = T // 128  # 8 partition tiles along seq dim
NP = H // 2  # 6 head pairs (pair p = heads 2p, 2p+1 living in qhT/khT[p])


def _ldw_sig(inst):
    return (
        str(inst.ins[0]),
        str(inst.tile_position),
        str(inst.tile_size),
        str(inst.perf_mode),
        str(inst.is_transpose),
    )


def _row_range(inst):
    tp = inst.tile_position
    ts = inst.tile_size
    r0 = tp[0] if tp else 0
    rs = ts[0] if ts else 128
    return (r0, r0 + rs)


def _elide_redundant_ldweights(nc):
    """Drop Ldweights whose weights AP matches the last load into the same PE
    row range, with no overlapping load in between (matmults carry
    ldweights=False post-legalize, so walrus reuses the PE array contents).
    Tracked per row-group so row-tiled matmul pairs can ping-pong without
    reloading. Waits/deps of dropped loads move to the next PE instruction."""
    removed = 0
    for b in nc.main_func.blocks:
        insts = list(b.instructions)
        keep = []
        last = {}  # (row0, row1) -> sig
        pending = None
        for inst in insts:
            if isinstance(inst, mybir.InstLdweights):
                rr = _row_range(inst)
                s = _ldw_sig(inst)
                if last.get(rr) == s:
                    pending = inst
                    removed += 1
                    continue
                # invalidate overlapping row ranges
                for k in [k for k in last if not (k[1] <= rr[0] or k[0] >= rr[1])]:
                    del last[k]
                last[rr] = s
            elif isinstance(inst, mybir.InstMatmult):
                if pending is not None:
                    si = pending.sync_info
                    if si is not None and (len(si.on_wait) or len(si.on_update)):
                        mi = inst.sync_info
                        ow = list(si.on_wait)
                        ou = list(si.on_update)
                        if mi is not None:
                            ow = list(mi.on_wait) + ow
                            ou = list(mi.on_update) + ou
                        inst.sync_info = mybir.SyncInfo(on_wait=ow, on_update=ou)
                    inst.merge_dependencies_from(pending)
                    pending = None
            elif getattr(inst, "engine", None) == mybir.EngineType.PE:
                last.clear()
                if pending is not None:
                    inst.merge_dependencies_from(pending)
                    pending = None
            keep.append(inst)
        if len(keep) != len(insts):
            del b.instructions[:]
            b.instructions.extend(keep)
    return removed


def _build():
    nc = bacc.Bacc("TRN2", target_bir_lowering=False, debug=False)

    qT = nc.declare_dram_parameter("qT", [E, T], BF, isOutput=False)
    kT = nc.declare_dram_parameter("kT", [E, T], BF, isOutput=False)
    vT = nc.declare_dram_parameter("vT", [E, T], BF, isOutput=False)
    WqT = nc.declare_dram_parameter("WqT", [E, E], BF, isOutput=False)
    WkT = nc.declare_dram_parameter("WkT", [E, E], BF, isOutput=False)
    WvT = nc.declare_dram_parameter("WvT", [E, E], BF, isOutput=False)
    WoT = nc.declare_dram_parameter("WoT", [E, E], BF, isOutput=False)
    selD = nc.declare_dram_parameter("selD", [97, 384], BF, isOutput=False)
    out = nc.declare_dram_parameter("out", [T, E], F32, isOutput=True)

    EXP = mybir.ActivationFunctionType.Exp

    with tile.TileContext(nc) as tc:
        with (
            tc.tile_pool(name="persist", bufs=1) as pp,
            tc.tile_pool(name="xin", bufs=2) as xp,
            tc.tile_pool(name="w", bufs=2) as wp,
            tc.tile_pool(name="exps", bufs=2) as ep,
            tc.tile_pool(name="dn", bufs=1) as dn,
            tc.tile_pool(name="ob", bufs=2) as op,
            tc.tile_pool(name="pmm", bufs=1, space="PSUM") as pmm,
            tc.tile_pool(name="pscore", bufs=1, space="PSUM") as psc,
            tc.tile_pool(name="pctx", bufs=2, space="PSUM") as pcx,
        ):
            # ---- persistent sbuf tensors ----
            qhT = [pp.tile([128, T], BF, name=f"qhT{i}") for i in range(NE)]
            khT = [pp.tile([128, T], BF, name=f"khT{i}") for i in range(NE)]
            vh1 = [pp.tile([128, H * HD1], BF, name=f"vh1_{i}") for i in range(NT)]
            # mgP[p]: unnormalized ctx (heads 2p rows 0-63 / 2p+1 rows 64-127),
            # normalized IN PLACE before the output projection.
            mgP = [pp.tile([128, T], BF, name=f"mgP{p}") for p in range(NE)]
            sel = pp.tile([97, 384], BF, name="sel")
            scrA = pp.tile([97, 512], F32, name="scrA")
            scrB = pp.tile([33, 512], F32, name="scrB")
            # den/rcp tiles are shared between head groups g=0/1 via a bufs=1
            # pool: group 1's memset WAR-waits on group 0's last reader.
            _den_cache = {}

            def get_den(g):
                if g not in _den_cache:
                    dA = [
                        dn.tile([97, 512], F32, tag=f"dA{qb}", name=f"dA{g}_{qb}")
                        for qb in range(2)
                    ]
                    dB = [
                        dn.tile([33, 512], F32, tag=f"dB{qb}", name=f"dB{g}_{qb}")
                        for qb in range(2)
                    ]
                    rA = [
                        dn.tile([97, 512], BF, tag=f"rA{qb}", name=f"rA{g}_{qb}")
                        for qb in range(2)
                    ]
                    rB = [
                        dn.tile([33, 512], BF, tag=f"rB{qb}", name=f"rB{g}_{qb}")
                        for qb in range(2)
                    ]
                    for qb in range(2):
                        nc.vector.memset(dA[qb][:], 1.0)
                        nc.vector.memset(dB[qb][:], 1.0)
                    _den_cache[g] = (dA, dB, rA, rB)
                return _den_cache[g]

            def dma_in_chunks(dst, src, nch=2, eng=None):
                # each dma_start costs ~0.5us on its issuing sequencer, so
                # chunk sparingly and spread issues across SP and Activation
                eng = eng or nc.sync
                p = dst.shape[0]
                step = p // nch
                for c in range(nch):
                    eng.dma_start(
                        dst[c * step : (c + 1) * step, :],
                        src[c * step : (c + 1) * step, :],
                    )

            def load6(dram, pool, tag_prefix, cols, nch=1, eng=None):
                ts = []
                for i in range(NE):
                    t_ = pool.tile(
                        [128, cols], BF, tag=f"{tag_prefix}{i}", name=f"{tag_prefix}{i}"
                    )
                    dma_in_chunks(t_, dram[i * 128 : (i + 1) * 128, :], nch, eng)
                    ts.append(t_)
                return ts

            # ---- upfront DMA issue (interleaved so first-needed lands
            # first; weights on the sync sequencer, activations on scalar)
            xtq, wtq, xtk, wtk = [], [], [], []
            for i in range(NE):
                nch = 4 if i == 0 else 2
                w_ = wp.tile([128, E], BF, tag=f"w{i}", name=f"wq{i}")
                dma_in_chunks(w_, WqT[i * 128 : (i + 1) * 128, :], nch, nc.sync)
                wtq.append(w_)
                x_ = xp.tile([128, T], BF, tag=f"x{i}", name=f"xq{i}")
                dma_in_chunks(x_, qT[i * 128 : (i + 1) * 128, :], nch, nc.scalar)
                xtq.append(x_)
                w2 = wp.tile([128, E], BF, tag=f"w{i}", name=f"wk{i}")
                dma_in_chunks(w2, WkT[i * 128 : (i + 1) * 128, :], nch, nc.sync)
                wtk.append(w2)
                x2 = xp.tile([128, T], BF, tag=f"x{i}", name=f"xk{i}")
                dma_in_chunks(x2, kT[i * 128 : (i + 1) * 128, :], nch, nc.scalar)
                xtk.append(x2)
            # V inputs/weights get their own tags: they are consumed (during
            # head pairs 0-1) long before the last Q/K-proj matmuls that a
            # shared-tag WAR dependency would wait on.
            xv = load6(vT, pp, "xv", T)
            wv = load6(WvT, pp, "wv", E)
            nc.sync.dma_start(sel[:], selD[:, :])
            for tt in range(NT):
                v_ = vh1[tt][:].rearrange("p (h d) -> p h d", d=HD1)
                nc.vector.memset(v_[:, :, DH:HD1], 1.0)

            # ---- emission helpers (generators yield (ns_estimate) per chunk)
            def proj_qk(xt, wt, dst, oc):
                # dst[oc][o, t] = sum_i W[i, o]^T x[i, t]; i-outer so each
                # weight block is loaded once and serves both 512-col halves
                ps = pmm.tile([128, T], F32, tag="mm", name=f"pj{oc}")
                for i in range(NE):
                    for half in range(2):
                        c0 = half * 512
                        nc.tensor.matmul(
                            ps[:, c0 : c0 + 512],
                            wt[i][:, oc * 128 : (oc + 1) * 128],
                            xt[i][:, c0 : c0 + 512],
                            start=(i == 0),
                            stop=(i == NE - 1),
                            skip_group_check=True,
                        )
                    if i == 2:
                        yield 1600
                # split the drain copy across DVE and the idle GpSimd so the
                # single-buffered psum frees fast
                nc.vector.tensor_copy(dst[oc][:, 0:512], ps[:, 0:512])
                nc.vector.tensor_copy(dst[oc][:, 512:1024], ps[:, 512:1024])
                yield 1600

            def proj_v(tt):
                # vh[t, (h d)] = sum_i vT[i, t]^T WvT[i, (h d)]
                v_ = vh1[tt][:].rearrange("p (h d) -> p h d", d=HD1)
                ps = pmm.tile([128, E], F32, tag="mm", name=f"pv{tt}")
                for i in range(NE):
                    for half, cw in ((0, 512), (1, 256)):
                        c0 = half * 512
                        nc.tensor.matmul(
                            ps[:, c0 : c0 + cw],
                            xv[i][:, tt * 128 : (tt + 1) * 128],
                            wv[i][:, c0 : c0 + cw],
                            start=(i == 0),
                            stop=(i == NE - 1),
                            skip_group_check=True,
                        )
                    if i == 2:
                        yield 1300
                nc.vector.tensor_copy(
                    v_[:, 0:6, 0:DH],
                    ps[:, 0:384].rearrange("p (h d) -> p h d", d=DH),
                )
                nc.vector.tensor_copy(
                    v_[:, 6:H, 0:DH],
                    ps[:, 384:E].rearrange("p (h d) -> p h d", d=DH),
                )
                yield 1300

            def scores_round(p, kt, esA, esB):
                # paired row-tiled scores: head 2p in PE rows 0-63,
                # head 2p+1 in rows 64-127, concurrent per qb; one exp
                # instruction per head over the full 1024 queries.
                pa = psc.tile([128, T], F32, tag="sA", name=f"sA{p}_{kt}")
                pb = psc.tile([128, T], F32, tag="sB", name=f"sB{p}_{kt}")
                for qb in range(2):
                    c0 = qb * 512
                    nc.tensor.matmul(
                        pa[:, c0 : c0 + 512],
                        khT[p][0:DH, kt * 128 : (kt + 1) * 128],
                        qhT[p][0:DH, c0 : c0 + 512],
                        start=True,
                        stop=True,
                        skip_group_check=True,
                    )
                    nc.tensor.matmul(
                        pb[:, c0 : c0 + 512],
                        khT[p][DH:128, kt * 128 : (kt + 1) * 128],
                        qhT[p][DH:128, c0 : c0 + 512],
                        start=True,
                        stop=True,
                        skip_group_check=True,
                    )
                nc.scalar.activation(esA[kt][:], pa[:], EXP, scale=0.125)
                nc.scalar.activation(esB[kt][:], pb[:], EXP, scale=0.125)

            def ctx_head(h, es):
                # unnormalized ctx + denominator via the ones column, yields per kt
                pcs = [
                    pcx.tile([HD1, 512], F32, tag="ctx", name=f"pc{h}_{qb}")
                    for qb in range(2)
                ]
                for kt in range(NT):
                    for qb in range(2):
                        nc.tensor.matmul(
                            pcs[qb][:],
                            vh1[kt][:, h * HD1 : (h + 1) * HD1],
                            es[kt][:, qb * 512 : (qb + 1) * 512],
                            start=(kt == 0),
                            stop=(kt == NT - 1),
                            skip_group_check=True,
                        )
                    yield 550
                g, r = h // 6, h % 6
                p2, half = h // 2, h % 2
                dA, dB, rA, rB = get_den(g)
                dent = dA if r < 4 else dB
                drow = 32 * r if r < 4 else 32 * (r - 4)
                for qb in range(2):
                    nc.vector.tensor_copy(
                        mgP[p2][half * DH : (half + 1) * DH, qb * 512 : (qb + 1) * 512],
                        pcs[qb][0:DH, :],
                    )
                    nc.vector.tensor_copy(
                        dent[qb][drow : drow + 1, :], pcs[qb][DH:HD1, :]
                    )
                if r == 3:
                    for qb in range(2):
                        recip(rA[qb], dA[qb], scrA)
                if r == 4:
                    norm_pairs(g, (0, 1))
                if r == 5:
                    for qb in range(2):
                        recip(rB[qb], dB[qb], scrB)
                    norm_pairs(g, (2,))
                yield 800

            def recip(dst, den_t, scr):
                nc.vector.reciprocal_approx_fast(scr[:], den_t[:])
                nc.vector.tensor_copy(dst[:], scr[:])

            def norm_pairs(g, js):
                # broadcast 1/den to 64 rows/head via PE, normalize mgP in place
                _, _, rA, rB = get_den(g)
                for j in js:
                    p = g * 3 + j
                    for qb in range(2):
                        bcps = pmm.tile([128, 512], F32, tag="mm", name=f"bc{p}_{qb}")
                        if j < 2:
                            lhsT = sel[0:97, j * 128 : (j + 1) * 128]
                            rhs = rA[qb][:]
                        else:
                            lhsT = sel[0:33, 256:384]
                            rhs = rB[qb][:]
                        nc.tensor.matmul(bcps[:], lhsT, rhs, start=True, stop=True)
                        nc.vector.tensor_mul(
                            mgP[p][:, qb * 512 : (qb + 1) * 512],
                            mgP[p][:, qb * 512 : (qb + 1) * 512],
                            bcps[:],
                        )

            # ---- the interleaved schedule ----
            # filler generators consumed a few PE-chunks per score round
            def chain(gens):
                for gg in gens:
                    yield from gg

            esd = {}

            def es_tiles(p):
                # bufs=2 rotation: pair p and p-2 share a buffer. ctx(p-2) is
                # fully emitted during pair p-1 (lag-1 staggering), so the WAR
                # dep of exp(p) on ctx(p-2) points backward in program order.
                A = [
                    ep.tile([128, T], BF, tag=f"eA{kt}", name=f"eA{p}_{kt}")
                    for kt in range(NT)
                ]
                Bt = [
                    ep.tile([128, T], BF, tag=f"eB{kt}", name=f"eB{p}_{kt}")
                    for kt in range(NT)
                ]
                return A, Bt

            # prologue: only pair 0's projections; everything else is filler
            for gen in (proj_qk(xtq, wtq, qhT, 0), proj_qk(xtk, wtk, khT, 0)):
                for _ in gen:
                    pass

            for p in range(NP):
                esA, esB = es_tiles(p)
                esd[2 * p] = esA
                esd[2 * p + 1] = esB
                # DMA-independent work (projections of already-loaded Q/K)
                # leads each chain; V waits for its own late-arriving DMAs.
                gens = []
                if p + 1 < NP:
                    gens.append(proj_qk(xtq, wtq, qhT, p + 1))
                    gens.append(proj_qk(xtk, wtk, khT, p + 1))
                if p == 0:
                    gens.extend(proj_v(tt) for tt in range(5))
                elif p == 1:
                    gens.extend(proj_v(tt) for tt in range(5, NT))
                if p > 0:
                    hA, hB = 2 * (p - 1), 2 * (p - 1) + 1
                    gens.append(ctx_head(hA, esd[hA]))
                    gens.append(ctx_head(hB, esd[hB]))
                fil = chain(gens)
                for kt in range(NT):
                    scores_round(p, kt, esA, esB)
                    # drain ~one ACT-round worth of filler to keep the PE
                    # queue deep while the exps grind
                    budget = 2100
                    while fil is not None and budget > 0:
                        try:
                            budget -= next(fil)
                        except StopIteration:
                            fil = None
                if fil is not None:
                    for _ in fil:
                        pass

            # epilogue: ctx for pair 5
            for h in range(2 * (NP - 1), H):
                for _ in ctx_head(h, esd[h]):
                    pass

            # Wo load late: reuses w{i} tags once K-projections are done
            wo = load6(WoT, wp, "w", E)

            # ---- output projection (double-buffered via the idle score
            # psum banks; i-outer so each mgP block loads once) ----
            for tt in range(NT):
                ob = op.tile([128, E], F32, tag="ob", name=f"ob{tt}")
                po = psc.tile(
                    [128, E], F32, tag=("sA" if tt % 2 == 0 else "sB"), name=f"po{tt}"
                )
                for p in range(NE):
                    for half, cw in ((0, 512), (1, 256)):
                        c0 = half * 512
                        nc.tensor.matmul(
                            po[:, c0 : c0 + cw],
                            mgP[p][:, tt * 128 : (tt + 1) * 128],
                            wo[p][:, c0 : c0 + cw],
                            start=(p == 0),
                            stop=(p == NE - 1),
                            skip_group_check=True,
                        )
                nc.vector.tensor_copy(ob[:], po[:])
                # 4 chunks, issues split across both DGE sequencers: the last
                # tile's 192KB would otherwise ride one ring for ~8.5us of tail
                for c in range(4):
                    eng = nc.sync if c < 2 else nc.scalar
                    eng.dma_start(
                        out[tt * 128 + c * 32 : tt * 128 + (c + 1) * 32, :],
                        ob[c * 32 : (c + 1) * 32, :],
                    )

    _elide_redundant_ldweights(nc)
    nc.finalize()
    return nc


_NC = None
TRACE = False
LAST_RESULT = None


def _get_nc():
    global _NC
    if _NC is None:
        _NC = _build()
    return _NC


def kernel(**inputs):
    q = np.asarray(inputs["q"], dtype=np.float32)
    k = np.asarray(inputs["k"], dtype=np.float32)
    v = np.asarray(inputs["v"], dtype=np.float32)
    w = {
        n: np.ascontiguousarray(np.asarray(inputs[n], dtype=np.float32).T).astype(F16)
        for n in ("Wq", "Wk", "Wv", "Wo")
    }
    sel = np.zeros((97, 384), dtype=F16)
    for j in range(3):
        sel[(32 * 2 * j) % 128, j * 128 : j * 128 + 64] = 1.0
        sel[(32 * (2 * j + 1)) % 128, j * 128 + 64 : (j + 1) * 128] = 1.0

    nc = _get_nc()
    in_maps = []
    for b in range(B):
        in_maps.append({
            "qT": np.ascontiguousarray(q[b].T).astype(F16),
            "kT": np.ascontiguousarray(k[b].T).astype(F16),
            "vT": np.ascontiguousarray(v[b].T).astype(F16),
            "WqT": w["Wq"],
            "WkT": w["Wk"],
            "WvT": w["Wv"],
            "WoT": w["Wo"],
            "selD": sel,
        })
    res = run_bass_kernel_spmd(nc, in_maps, list(range(B)), trace=TRACE)
    global LAST_RESULT
    LAST_RESULT = res
    return np.stack(
        [np.asarray(res.results[b]["out"], dtype=np.float32) for b in range(B)], axis=0
    )


# revision 37
# speedup vs baseline: 1.0124x; 1.0008x over previous
import numpy as np

import concourse.bass as bass
import concourse.bacc as bacc
import concourse.mybir as mybir
import concourse.tile as tile
from concourse.bass_utils import run_bass_kernel_spmd

F16 = np.float16
F32 = mybir.dt.float32
BF = mybir.dt.float16

B = 8
T = 1024
E = 768
H = 12
DH = 64
HD1 = DH + 1  # head dim + ones column for softmax denominator
NE = E // 128  # 6 partition tiles along embed dim
NT = T // 128  # 8 partition tiles along seq dim
NP = H // 2  # 6 head pairs (pair p = heads 2p, 2p+1 living in qhT/khT[p])


def _ldw_sig(inst):
    return (
        str(inst.ins[0]),
        str(inst.tile_position),
        str(inst.tile_size),
        str(inst.perf_mode),
        str(inst.is_transpose),
    )


def _row_range(inst):
    tp = inst.tile_position
    ts = inst.tile_size
    r0 = tp[0] if tp else 0
    rs = ts[0] if ts else 128
    return (r0, r0 + rs)


def _elide_redundant_ldweights(nc):
    """Drop Ldweights whose weights AP matches the last load into the same PE
    row range, with no overlapping load in between (matmults carry
    ldweights=False post-legalize, so walrus reuses the PE array contents).
    Tracked per row-group so row-tiled matmul pairs can ping-pong without
    reloading. Waits/deps of dropped loads move to the next PE instruction."""
    removed = 0
    for b in nc.main_func.blocks:
        insts = list(b.instructions)
        keep = []
        last = {}  # (row0, row1) -> sig
        pending = None
        for inst in insts:
            if isinstance(inst, mybir.InstLdweights):
                rr = _row_range(inst)
                s = _ldw_sig(inst)
                if last.get(rr) == s:
                    pending = inst
                    removed += 1
                    continue
                # invalidate overlapping row ranges
                for k in [k for k in last if not (k[1] <= rr[0] or k[0] >= rr[1])]:
                    del last[k]
                last[rr] = s
            elif isinstance(inst, mybir.InstMatmult):
                if pending is not None:
                    si = pending.sync_info
                    if si is not None and (len(si.on_wait) or len(si.on_update)):
                        mi = inst.sync_info
                        ow = list(si.on_wait)
                        ou = list(si.on_update)
                        if mi is not None:
                            ow = list(mi.on_wait) + ow
                            ou = list(mi.on_update) + ou
                        inst.sync_info = mybir.SyncInfo(on_wait=ow, on_update=ou)
                    inst.merge_dependencies_from(pending)
                    pending = None
            elif getattr(inst, "engine", None) == mybir.EngineType.PE:
                last.clear()
                if pending is not None:
                    inst.merge_dependencies_from(pending)
                    pending = None
            keep.append(inst)
        if len(keep) != len(insts):
            del b.instructions[:]
            b.instructions.extend(keep)
    return removed


def _build():
    nc = bacc.Bacc("TRN2", target_bir_lowering=False, debug=False)

    qT = nc.declare_dram_parameter("qT", [E, T], BF, isOutput=False)
    kT = nc.declare_dram_parameter("kT", [E, T], BF, isOutput=False)
    vT = nc.declare_dram_parameter("vT", [E, T], BF, isOutput=False)
    WqT = nc.declare_dram_parameter("WqT", [E, E], BF, isOutput=False)
    WkT = nc.declare_dram_parameter("WkT", [E, E], BF, isOutput=False)
    WvT = nc.declare_dram_parameter("WvT", [E, E], BF, isOutput=False)
    WoT = nc.declare_dram_parameter("WoT", [E, E], BF, isOutput=False)
    selD = nc.declare_dram_parameter("selD", [97, 384], BF, isOutput=False)
    out = nc.declare_dram_parameter("out", [T, E], F32, isOutput=True)

    EXP = mybir.ActivationFunctionType.Exp

    with tile.TileContext(nc) as tc:
        with (
            tc.tile_pool(name="persist", bufs=1) as pp,
            tc.tile_pool(name="xin", bufs=2) as xp,
            tc.tile_pool(name="w", bufs=2) as wp,
            tc.tile_pool(name="exps", bufs=2) as ep,
            tc.tile_pool(name="dn", bufs=1) as dn,
            tc.tile_pool(name="ob", bufs=2) as op,
            tc.tile_pool(name="pmm", bufs=1, space="PSUM") as pmm,
            tc.tile_pool(name="pscore", bufs=1, space="PSUM") as psc,
            tc.tile_pool(name="pctx", bufs=2, space="PSUM") as pcx,
        ):
            # ---- persistent sbuf tensors ----
            qhT = [pp.tile([128, T], BF, name=f"qhT{i}") for i in range(NE)]
            khT = [pp.tile([128, T], BF, name=f"khT{i}") for i in range(NE)]
            vh1 = [pp.tile([128, H * HD1], BF, name=f"vh1_{i}") for i in range(NT)]
            # mgP[p]: unnormalized ctx (heads 2p rows 0-63 / 2p+1 rows 64-127),
            # normalized IN PLACE before the output projection.
            mgP = [pp.tile([128, T], BF, name=f"mgP{p}") for p in range(NE)]
            sel = pp.tile([97, 384], BF, name="sel")
            scrA = pp.tile([97, 512], F32, name="scrA")
            scrB = pp.tile([33, 512], F32, name="scrB")
            # den/rcp tiles are shared between head groups g=0/1 via a bufs=1
            # pool: group 1's memset WAR-waits on group 0's last reader.
            _den_cache = {}

            def get_den(g):
                if g not in _den_cache:
                    dA = [
                        dn.tile([97, 512], F32, tag=f"dA{qb}", name=f"dA{g}_{qb}")
                        for qb in range(2)
                    ]
                    dB = [
                        dn.tile([33, 512], F32, tag=f"dB{qb}", name=f"dB{g}_{qb}")
                        for qb in range(2)
                    ]
                    rA = [
                        dn.tile([97, 512], BF, tag=f"rA{qb}", name=f"rA{g}_{qb}")
                        for qb in range(2)
                    ]
                    rB = [
                        dn.tile([33, 512], BF, tag=f"rB{qb}", name=f"rB{g}_{qb}")
                        for qb in range(2)
                    ]
                    for qb in range(2):
                        nc.vector.memset(dA[qb][:], 1.0)
                        nc.vector.memset(dB[qb][:], 1.0)
                    _den_cache[g] = (dA, dB, rA, rB)
                return _den_cache[g]

            def dma_in_chunks(dst, src, nch=2, eng=None):
                # each dma_start costs ~0.5us on its issuing sequencer, so
                # chunk sparingly and spread issues across SP and Activation
                eng = eng or nc.sync
                p = dst.shape[0]
                step = p // nch
                for c in range(nch):
                    eng.dma_start(
                        dst[c * step : (c + 1) * step, :],
                        src[c * step : (c + 1) * step, :],
                    )

            def load6(dram, pool, tag_prefix, cols, nch=1, eng=None):
                ts = []
                for i in range(NE):
                    t_ = pool.tile(
                        [128, cols], BF, tag=f"{tag_prefix}{i}", name=f"{tag_prefix}{i}"
                    )
                    dma_in_chunks(t_, dram[i * 128 : (i + 1) * 128, :], nch, eng)
                    ts.append(t_)
                return ts

            # ---- upfront DMA issue (interleaved so first-needed lands
            # first; weights on the sync sequencer, activations on scalar)
            xtq, wtq, xtk, wtk = [], [], [], []
            for i in range(NE):
                nch = 2
                w_ = wp.tile([128, E], BF, tag=f"w{i}", name=f"wq{i}")
                dma_in_chunks(w_, WqT[i * 128 : (i + 1) * 128, :], nch, nc.sync)
                wtq.append(w_)
                x_ = xp.tile([128, T], BF, tag=f"x{i}", name=f"xq{i}")
                dma_in_chunks(x_, qT[i * 128 : (i + 1) * 128, :], nch, nc.scalar)
                xtq.append(x_)
                w2 = wp.tile([128, E], BF, tag=f"w{i}", name=f"wk{i}")
                dma_in_chunks(w2, WkT[i * 128 : (i + 1) * 128, :], nch, nc.sync)
                wtk.append(w2)
                x2 = xp.tile([128, T], BF, tag=f"x{i}", name=f"xk{i}")
                dma_in_chunks(x2, kT[i * 128 : (i + 1) * 128, :], nch, nc.scalar)
                xtk.append(x2)
            # V inputs/weights get their own tags: they are consumed (during
            # head pairs 0-1) long before the last Q/K-proj matmuls that a
            # shared-tag WAR dependency would wait on.
            xv = load6(vT, pp, "xv", T)
            wv = load6(WvT, pp, "wv", E)
            nc.sync.dma_start(sel[:], selD[:, :])
            for tt in range(NT):
                v_ = vh1[tt][:].rearrange("p (h d) -> p h d", d=HD1)
                nc.vector.memset(v_[:, :, DH:HD1], 1.0)

            # ---- emission helpers (generators yield (ns_estimate) per chunk)
            def proj_qk(xt, wt, dst, oc):
                # dst[oc][o, t] = sum_i W[i, o]^T x[i, t]; i-outer so each
                # weight block is loaded once and serves both 512-col halves
                ps = pmm.tile([128, T], F32, tag="mm", name=f"pj{oc}")
                for i in range(NE):
                    for half in range(2):
                        c0 = half * 512
                        nc.tensor.matmul(
                            ps[:, c0 : c0 + 512],
                            wt[i][:, oc * 128 : (oc + 1) * 128],
                            xt[i][:, c0 : c0 + 512],
                            start=(i == 0),
                            stop=(i == NE - 1),
                            skip_group_check=True,
                        )
                    if i == 2:
                        yield 1600
                # split the drain copy across DVE and the idle GpSimd so the
                # single-buffered psum frees fast
                nc.vector.tensor_copy(dst[oc][:, 0:512], ps[:, 0:512])
                nc.vector.tensor_copy(dst[oc][:, 512:1024], ps[:, 512:1024])
                yield 1600

            def proj_v(tt):
                # vh[t, (h d)] = sum_i vT[i, t]^T WvT[i, (h d)]
                v_ = vh1[tt][:].rearrange("p (h d) -> p h d", d=HD1)
                ps = pmm.tile([128, E], F32, tag="mm", name=f"pv{tt}")
                for i in range(NE):
                    for half, cw in ((0, 512), (1, 256)):
                        c0 = half * 512
                        nc.tensor.matmul(
                            ps[:, c0 : c0 + cw],
                            xv[i][:, tt * 128 : (tt + 1) * 128],
                            wv[i][:, c0 : c0 + cw],
                            start=(i == 0),
                            stop=(i == NE - 1),
                            skip_group_check=True,
                        )
                    if i == 2:
                        yield 1300
                nc.vector.tensor_copy(
                    v_[:, 0:6, 0:DH],
                    ps[:, 0:384].rearrange("p (h d) -> p h d", d=DH),
                )
                nc.vector.tensor_copy(
                    v_[:, 6:H, 0:DH],
                    ps[:, 384:E].rearrange("p (h d) -> p h d", d=DH),
                )
                yield 1300

            def scores_round(p, kt, esA, esB):
                # paired row-tiled scores: head 2p in PE rows 0-63,
                # head 2p+1 in rows 64-127, concurrent per qb; one exp
                # instruction per head over the full 1024 queries.
                pa = psc.tile([128, T], F32, tag="sA", name=f"sA{p}_{kt}")
                pb = psc.tile([128, T], F32, tag="sB", name=f"sB{p}_{kt}")
                for qb in range(2):
                    c0 = qb * 512
                    nc.tensor.matmul(
                        pa[:, c0 : c0 + 512],
                        khT[p][0:DH, kt * 128 : (kt + 1) * 128],
                        qhT[p][0:DH, c0 : c0 + 512],
                        start=True,
                        stop=True,
                        skip_group_check=True,
                    )
                    nc.tensor.matmul(
                        pb[:, c0 : c0 + 512],
                        khT[p][DH:128, kt * 128 : (kt + 1) * 128],
                        qhT[p][DH:128, c0 : c0 + 512],
                        start=True,
                        stop=True,
                        skip_group_check=True,
                    )
                nc.scalar.activation(esA[kt][:], pa[:], EXP, scale=0.125)
                nc.scalar.activation(esB[kt][:], pb[:], EXP, scale=0.125)

            def ctx_head(h, es):
                # unnormalized ctx + denominator via the ones column, yields per kt
                pcs = [
                    pcx.tile([HD1, 512], F32, tag="ctx", name=f"pc{h}_{qb}")
                    for qb in range(2)
                ]
                for kt in range(NT):
                    for qb in range(2):
                        nc.tensor.matmul(
                            pcs[qb][:],
                            vh1[kt][:, h * HD1 : (h + 1) * HD1],
                            es[kt][:, qb * 512 : (qb + 1) * 512],
                            start=(kt == 0),
                            stop=(kt == NT - 1),
                            skip_group_check=True,
                        )
                    yield 550
                g, r = h // 6, h % 6
                p2, half = h // 2, h % 2
                dA, dB, rA, rB = get_den(g)
                dent = dA if r < 4 else dB
                drow = 32 * r if r < 4 else 32 * (r - 4)
                for qb in range(2):
                    nc.vector.tensor_copy(
                        mgP[p2][half * DH : (half + 1) * DH, qb * 512 : (qb + 1) * 512],
                        pcs[qb][0:DH, :],
                    )
                    nc.vector.tensor_copy(
                        dent[qb][drow : drow + 1, :], pcs[qb][DH:HD1, :]
                    )
                if r == 3:
                    for qb in range(2):
                        recip(rA[qb], dA[qb], scrA)
                if r == 4:
                    norm_pairs(g, (0, 1))
                if r == 5:
                    for qb in range(2):
                        recip(rB[qb], dB[qb], scrB)
                    norm_pairs(g, (2,))
                yield 800

            def recip(dst, den_t, scr):
                nc.vector.reciprocal_approx_fast(scr[:], den_t[:])
                nc.vector.tensor_copy(dst[:], scr[:])

            def norm_pairs(g, js):
                # broadcast 1/den to 64 rows/head via PE, normalize mgP in place
                _, _, rA, rB = get_den(g)
                for j in js:
                    p = g * 3 + j
                    for qb in range(2):
                        bcps = pmm.tile([128, 512], F32, tag="mm", name=f"bc{p}_{qb}")
                        if j < 2:
                            lhsT = sel[0:97, j * 128 : (j + 1) * 128]
                            rhs = rA[qb][:]
                        else:
                            lhsT = sel[0:33, 256:384]
                            rhs = rB[qb][:]
                        nc.tensor.matmul(bcps[:], lhsT, rhs, start=True, stop=True)
                        nc.vector.tensor_mul(
                            mgP[p][:, qb * 512 : (qb + 1) * 512],
                            mgP[p][:, qb * 512 : (qb + 1) * 512],
                            bcps[:],
                        )

            # ---- the interleaved schedule ----
            # filler generators consumed a few PE-chunks per score round
            def chain(gens):
                for gg in gens:
                    yield from gg

            esd = {}

            def es_tiles(p):
                # bufs=2 rotation: pair p and p-2 share a buffer. ctx(p-2) is
                # fully emitted during pair p-1 (lag-1 staggering), so the WAR
                # dep of exp(p) on ctx(p-2) points backward in program order.
                A = [
                    ep.tile([128, T], BF, tag=f"eA{kt}", name=f"eA{p}_{kt}")
                    for kt in range(NT)
                ]
                Bt = [
                    ep.tile([128, T], BF, tag=f"eB{kt}", name=f"eB{p}_{kt}")
                    for kt in range(NT)
                ]
                return A, Bt

            # prologue: only pair 0's projections; everything else is filler
            for gen in (proj_qk(xtq, wtq, qhT, 0), proj_qk(xtk, wtk, khT, 0)):
                for _ in gen:
                    pass

            for p in range(NP):
                esA, esB = es_tiles(p)
                esd[2 * p] = esA
                esd[2 * p + 1] = esB
                # DMA-independent work (projections of already-loaded Q/K)
                # leads each chain; V waits for its own late-arriving DMAs.
                gens = []
                if p + 1 < NP:
                    gens.append(proj_qk(xtq, wtq, qhT, p + 1))
                    gens.append(proj_qk(xtk, wtk, khT, p + 1))
                if p == 0:
                    gens.extend(proj_v(tt) for tt in range(5))
                elif p == 1:
                    gens.extend(proj_v(tt) for tt in range(5, NT))
                if p > 0:
                    hA, hB = 2 * (p - 1), 2 * (p - 1) + 1
                    gens.append(ctx_head(hA, esd[hA]))
                    gens.append(ctx_head(hB, esd[hB]))
                fil = chain(gens)
                for kt in range(NT):
                    scores_round(p, kt, esA, esB)
                    # drain ~one ACT-round worth of filler to keep the PE
                    # queue deep while the exps grind
                    budget = 2100
                    while fil is not None and budget > 0:
                        try:
                            budget -= next(fil)
                        except StopIteration:
                            fil = None
                if fil is not None:
                    for _ in fil:
                        pass

            # epilogue: ctx for pair 5
            for h in range(2 * (NP - 1), H):
                for _ in ctx_head(h, esd[h]):
                    pass

            # Wo load late: reuses w{i} tags once K-projections are done
            wo = load6(WoT, wp, "w", E)

            # ---- output projection (double-buffered via the idle score
            # psum banks; i-outer so each mgP block loads once) ----
            for tt in range(NT):
                ob = op.tile([128, E], F32, tag="ob", name=f"ob{tt}")
                po = psc.tile(
                    [128, E], F32, tag=("sA" if tt % 2 == 0 else "sB"), name=f"po{tt}"
                )
                for p in range(NE):
                    for half, cw in ((0, 512), (1, 256)):
                        c0 = half * 512
                        nc.tensor.matmul(
                            po[:, c0 : c0 + cw],
                            mgP[p][:, tt * 128 : (tt + 1) * 128],
                            wo[p][:, c0 : c0 + cw],
                            start=(p == 0),
                            stop=(p == NE - 1),
                            skip_group_check=True,
                        )
                nc.vector.tensor_copy(ob[:], po[:])
                # 4 chunks with issues split across the two idle sequencers:
                # the final tile's 192KB would otherwise ride one DMA ring for
                # ~8.5us of pure tail
                for c in range(4):
                    eng = nc.sync if c < 2 else nc.scalar
                    eng.dma_start(
                        out[tt * 128 + c * 32 : tt * 128 + (c + 1) * 32, :],
                        ob[c * 32 : (c + 1) * 32, :],
                    )

    _elide_redundant_ldweights(nc)
    nc.finalize()
    return nc


_NC = None
TRACE = False
LAST_RESULT = None


def _get_nc():
    global _NC
    if _NC is None:
        _NC = _build()
    return _NC


def kernel(**inputs):
    q = np.asarray(inputs["q"], dtype=np.float32)
    k = np.asarray(inputs["k"], dtype=np.float32)
    v = np.asarray(inputs["v"], dtype=np.float32)
    w = {
        n: np.ascontiguousarray(np.asarray(inputs[n], dtype=np.float32).T).astype(F16)
        for n in ("Wq", "Wk", "Wv", "Wo")
    }
    sel = np.zeros((97, 384), dtype=F16)
    for j in range(3):
        sel[(32 * 2 * j) % 128, j * 128 : j * 128 + 64] = 1.0
        sel[(32 * (2 * j + 1)) % 128, j * 128 + 64 : (j + 1) * 128] = 1.0

    nc = _get_nc()
    in_maps = []
    for b in range(B):
        in_maps.append({
            "qT": np.ascontiguousarray(q[b].T).astype(F16),
            "kT": np.ascontiguousarray(k[b].T).astype(F16),
            "vT": np.ascontiguousarray(v[b].T).astype(F16),
            "WqT": w["Wq"],
            "WkT": w["Wk"],
            "WvT": w["Wv"],
            "WoT": w["Wo"],
            "selD": sel,
        })
    res = run_bass_kernel_spmd(nc, in_maps, list(range(B)), trace=TRACE)
    global LAST_RESULT
    LAST_RESULT = res
    return np.stack(
        [np.asarray(res.results[b]["out"], dtype=np.float32) for b in range(B)], axis=0
    )
